# revision 43
# baseline (speedup 1.0000x reference)
"""Trainium2 Bass kernel for nn_EnhanceDiversityFeatureExtracition.

Computes  loss = mean((x-y)^2) + ALPHA * diversity_reg(conv_w)
where diversity_reg builds a 64x64 Gram matrix of the F=64 slices
conv_w[:, :, i, :] (each flattened to a 786432-vector), normalizes it to
cosine similarities, and sums the entries with tau < sim <= 1 off the
diagonal.

Distribution (8 NeuronCores, SPMD):
  - x_batch / y_batch sharded on batch dim: 256 rows per core.
  - conv_w viewed as A = conv_w.reshape(262144, 192)  (row m = (o,c),
    col = f*3+k).  gram[i,j] = sum_m sum_k A[m,3i+k]*A[m,3j+k], so A is
    sharded along the 262144-row reduction axis: 32768 rows per core.
  - Each core returns the partial 192x192 C = A_shard^T A_shard (as a
    128x192 + 64x192 pair) and per-partition partial sums of (x-y)^2;
    the host sums partials, extracts gram[i,j] = sum_k C[3i+k,3j+k] and
    applies the tiny 64x64 masked-similarity epilogue.

On-core dataflow (v22b, tuned from traces):
  - ALL input DMAs ride the single Sync HWDGE ring: strict FIFO gives
    exact, program-order arrival.  Splitting A and x/y across the two
    HWDGE rings produced uncontrollable arbitration: whenever the x/y
    ring accumulated a queue it binged at full bandwidth and silenced
    the A ring for ~5us, starving the in-order matmul consumer.
  - The PE's warm rate (2 fp32r matmuls per 128-row strip, ~233ns) is
    slightly SLOWER than the solo A feed (~447GB/s = 213ns/strip), so
    the PE must idle for the x/y transfer time no matter what; the
    optimum is to take ALL of it up-front, before the PE has any work:
    x/y bulk first, then the whole A stream.  Uniform 32-strip blocks
    satisfy the bridge rule (7.5us of matmuls per block >= 6.8us
    next-block landing), so after one warm-up the PE runs gap-free and
    HAM never re-throttles -- measured ~11us more robust in the
    device's throttled state than sandwich schedules, which pay cold
    restarts.  Small final blocks trim the tail.
  - Per 128-row strip: 2 fp32r matmuls (moving width 256 for the
    full-rate mode; a bf16 variant measured 2.4x slower per strip)
    accumulate C into PSUM across all 256 strips.
  - MSE: DVE computes d = x-y, ACT computes Square(d) with per-chunk
    per-partition accumulate, all early; output DMAs ride the
    Activation ring, idle until then.
"""

import numpy as np

import concourse.bass as bass
import concourse.mybir as mybir
from concourse import bacc, tile
from concourse.bass_utils import run_bass_kernel_spmd

N_CORES = 8
B, D = 2048, 4096            # x_batch / y_batch
M, G = 262144, 192           # conv_w as (M, G); G = F*KW
F, KW = 64, 3
ROWS = B // N_CORES          # 256 batch rows per core
MC = M // N_CORES            # 32768 reduction rows per core
NSTRIP = MC // 128           # 256 strips of 128 rows per core

# A-block plan (strips per block): uniform 3MB blocks (a ramp of small
# first blocks violates the bridge rule after the x/y bulk -- the PE
# drains them faster than the next block lands and HAM re-throttles);
# small tail blocks trim the post-stream lag.
A_RAMP = []
A_MAIN = [32, 32, 32, 32, 32, 32, 32, 16, 8, 8]
A_PLAN = A_RAMP + A_MAIN
assert sum(A_PLAN) == NSTRIP

NXY = 2                      # MSE half-chunks per core
XYW = (ROWS * D) // (128 * NXY)   # 4096 floats per partition per half

ALPHA = 0.0005
TAU = 0.2

_prog = None


def _build() -> bass.Bass:
    nc = bacc.Bacc(None, target_bir_lowering=False)
    f32 = mybir.dt.float32
    f32r = mybir.dt.float32r

    xs = nc.dram_tensor("xs", [ROWS, D], f32, kind="ExternalInput")
    ys = nc.dram_tensor("ys", [ROWS, D], f32, kind="ExternalInput")
    aw = nc.dram_tensor("aw", [MC, G], f32r, kind="ExternalInput")
    c1_part = nc.dram_tensor("c1_part", [128, G], f32, kind="ExternalOutput")
    c2_part = nc.dram_tensor("c2_part", [F, G], f32, kind="ExternalOutput")
    sse_part = nc.dram_tensor("sse_part", [128, NXY], f32, kind="ExternalOutput")

    RW = 256                 # fp32r full-rate moving width
    PAD = RW - G             # 64 junk floats past each block's last strip
    MAXS = max(A_PLAN)

    with tile.TileContext(nc) as tc:
        with (
            tc.tile_pool(name="apool", bufs=4) as apool,
            tc.tile_pool(name="xpool", bufs=1) as xpool,
            tc.tile_pool(name="ypool", bufs=1) as ypool,
            tc.tile_pool(name="dpool", bufs=1) as dpool,
            tc.tile_pool(name="qpool", bufs=1) as qpool,
            tc.tile_pool(name="opool", bufs=1) as opool,
            tc.tile_pool(name="psum", bufs=1, space=bass.MemorySpace.PSUM) as psum,
        ):
            cps1 = psum.tile([128, RW], f32, tag="cps1")
            cps2 = psum.tile([F, RW], f32, tag="cps2")
            acc = opool.tile([128, NXY], f32)

            xv = xs[:].rearrange("(p t) d -> p (t d)", p=128)
            yv = ys[:].rearrange("(p t) d -> p (t d)", p=128)

            acts = []        # deferred (dtile, chunk-idx) Square-accums

            def emit_xy_bulk():
                # x and y as ONE 4MB DMA each (32KB per-partition
                # descriptors): every dma_start boundary costs ~1us of
                # per-engine descriptor-refill stall, so 2 transfers
                # instead of 8 shrink the PE's forced idle window by
                # several us.  MSE runs in two halves off the big tiles.
                # (Moving a y half later into the A stream was tried:
                # the PE lacks the backlog to absorb it - 4.6us gap plus
                # a HAM re-throttle, net wash.)
                xt = xpool.tile([128, NXY * XYW], f32)
                nc.sync.dma_start(xt[:], xv[:])
                yt = ypool.tile([128, NXY * XYW], f32)
                nc.sync.dma_start(yt[:], yv[:])
                for c in range(NXY):
                    dtile = dpool.tile([128, XYW], f32)
                    nc.vector.tensor_sub(
                        dtile[:], xt[:, c * XYW:(c + 1) * XYW],
                        yt[:, c * XYW:(c + 1) * XYW])
                    acts.append((dtile, c))

            # x/y bulk FIRST: the PE must idle for its transfer time
            # no matter what, and before the first A block it has no
            # work to lose; afterwards the A feed (~213ns/strip) stays
            # ahead of the PE (~233ns/strip) so the PE runs gap-free,
            # warm, with no mid-stream restart.  MSE pipelines in the
            # shadow of the A phase.
            emit_xy_bulk()

            ti = 0           # global strip counter
            row0 = 0
            for bi, ns in enumerate(A_PLAN):
                at = apool.tile([128, MAXS * G + PAD], f32r)
                src = aw[row0:row0 + 128 * ns].rearrange(
                    "(p t) g -> p (t g)", p=128)
                nc.sync.dma_start(at[:, :ns * G], src)
                nc.gpsimd.memset(
                    at[:, ns * G:ns * G + PAD].bitcast(f32), 0.0)
                row0 += 128 * ns

                for t in range(ns):
                    rhs = at[:, t * G:t * G + RW]
                    w1 = at[:, t * G:t * G + 128]
                    w2 = at[:, t * G + 128:t * G + G]
                    nc.tensor.matmul(
                        cps1[:], w1, rhs,
                        start=(ti == 0), stop=(ti == NSTRIP - 1),
                    )
                    nc.tensor.matmul(
                        cps2[:], w2, rhs,
                        start=(ti == 0), stop=(ti == NSTRIP - 1),
                    )
                    ti += 1

            # MSE squares on ACT run early (inputs land in the xy phase);
            # they must precede the C out-DMAs in the Activation queue,
            # which block on the final matmul.
            for dtile, c in acts:
                qtile = qpool.tile([128, XYW], f32)
                nc.scalar.activation(
                    qtile[:], dtile[:],
                    mybir.ActivationFunctionType.Square,
                    accum_out=acc[:, c:c + 1],
                )
            nc.scalar.dma_start(sse_part[:], acc[:])

            # C partials out on the Activation ring
            o1 = opool.tile([128, G], f32, tag="o1")
            nc.vector.tensor_copy(o1[:], cps1[:, :G])
            nc.scalar.dma_start(c1_part[:], o1[:])
            o2 = opool.tile([F, G], f32, tag="o2")
            nc.vector.tensor_copy(o2[:], cps2[:, :G])
            nc.scalar.dma_start(c2_part[:], o2[:])

    nc.finalize()
    return nc


def _get_prog() -> bass.Bass:
    global _prog
    if _prog is None:
        _prog = _build()
    return _prog


def _epilogue(C: np.ndarray, sse: float) -> np.ndarray:
    # gram[i,j] = sum_k C[3i+k, 3j+k]
    gram = np.einsum("ikjl,kl->ij", C.reshape(F, KW, F, KW), np.eye(KW))
    norms = np.sqrt(np.diag(gram))
    sim = gram / np.outer(norms, norms)
    mask = (sim > TAU) & (sim <= 1.0) & (~np.eye(F, dtype=bool))
    reg = sim[mask].sum()
    loss = sse / float(B * D) + ALPHA * reg
    return np.asarray(np.float32(loss))


def kernel(x_batch: np.ndarray, y_batch: np.ndarray, conv_w: np.ndarray) -> np.ndarray:
    nc = _get_prog()
    A = np.ascontiguousarray(conv_w.reshape(M, G))
    in_maps = []
    for c in range(N_CORES):
        in_maps.append({
            "xs": np.ascontiguousarray(x_batch[c * ROWS:(c + 1) * ROWS]),
            "ys": np.ascontiguousarray(y_batch[c * ROWS:(c + 1) * ROWS]),
            "aw": np.ascontiguousarray(A[c * MC:(c + 1) * MC]),
        })
    res = run_bass_kernel_spmd(nc, in_maps, core_ids=list(range(N_CORES))).results
    C = np.zeros((G, G), np.float64)
    sse = 0.0
    for r in res:
        C[:128] += r["c1_part"].astype(np.float64)
        C[128:] += r["c2_part"].astype(np.float64)
        sse += float(r["sse_part"].sum(dtype=np.float64))
    return _epilogue(C, sse)


# revision 45
# speedup vs baseline: 1.1141x; 1.1141x over previous
"""Trainium2 Bass kernel for nn_EnhanceDiversityFeatureExtracition.

Computes  loss = mean((x-y)^2) + ALPHA * diversity_reg(conv_w)
where diversity_reg builds a 64x64 Gram matrix of the F=64 slices
conv_w[:, :, i, :] (each flattened to a 786432-vector), normalizes it to
cosine similarities, and sums the entries with tau < sim <= 1 off the
diagonal.

Distribution (8 NeuronCores, SPMD):
  - x_batch / y_batch sharded on batch dim: 256 rows per core.
  - conv_w viewed as A = conv_w.reshape(262144, 192)  (row m = (o,c),
    col = f*3+k).  gram[i,j] = sum_m sum_k A[m,3i+k]*A[m,3j+k], so A is
    sharded along the 262144-row reduction axis: 32768 rows per core.
  - Each core returns the partial 192x192 C = A_shard^T A_shard (as a
    128x192 + 64x192 pair) and per-partition partial sums of (x-y)^2;
    the host sums partials, extracts gram[i,j] = sum_k C[3i+k,3j+k] and
    applies the tiny 64x64 masked-similarity epilogue.

On-core dataflow (v22b, tuned from traces):
  - ALL input DMAs ride the single Sync HWDGE ring: strict FIFO gives
    exact, program-order arrival.  Splitting A and x/y across the two
    HWDGE rings produced uncontrollable arbitration: whenever the x/y
    ring accumulated a queue it binged at full bandwidth and silenced
    the A ring for ~5us, starving the in-order matmul consumer.
  - The PE's warm rate (2 fp32r matmuls per 128-row strip, ~233ns) is
    slightly SLOWER than the solo A feed (~447GB/s = 213ns/strip), so
    the PE must idle for the x/y transfer time no matter what; the
    optimum is to take ALL of it up-front, before the PE has any work:
    x/y bulk first, then the whole A stream.  Uniform 32-strip blocks
    satisfy the bridge rule (7.5us of matmuls per block >= 6.8us
    next-block landing), so after one warm-up the PE runs gap-free and
    HAM never re-throttles -- measured ~11us more robust in the
    device's throttled state than sandwich schedules, which pay cold
    restarts.  Small final blocks trim the tail.
  - Per 128-row strip: 2 fp32r matmuls (moving width 256 for the
    full-rate mode; a bf16 variant measured 2.4x slower per strip)
    accumulate C into PSUM across all 256 strips.
  - MSE: DVE computes d = x-y, ACT computes Square(d) with per-chunk
    per-partition accumulate, all early; output DMAs ride the
    Activation ring, idle until then.
"""

import numpy as np

import concourse.bass as bass
import concourse.mybir as mybir
from concourse import bacc, tile
from concourse.bass_utils import run_bass_kernel_spmd

N_CORES = 8
B, D = 2048, 4096            # x_batch / y_batch
M, G = 262144, 192           # conv_w as (M, G); G = F*KW
F, KW = 64, 3
ROWS = B // N_CORES          # 256 batch rows per core
MC = M // N_CORES            # 32768 reduction rows per core
NSTRIP = MC // 128           # 256 strips of 128 rows per core

# A-block plan (strips per block): uniform 3MB blocks (a ramp of small
# first blocks violates the bridge rule after the x/y bulk -- the PE
# drains them faster than the next block lands and HAM re-throttles);
# small tail blocks trim the post-stream lag.
A_RAMP = []
A_MAIN = [32, 32, 32, 32, 32, 32, 32, 16, 8, 8]
A_PLAN = A_RAMP + A_MAIN
assert sum(A_PLAN) == NSTRIP

NXY = 2                      # MSE half-chunks per core
XYW = (ROWS * D) // (128 * NXY)   # 4096 floats per partition per half

ALPHA = 0.0005
TAU = 0.2

_prog = None


def _build() -> bass.Bass:
    nc = bacc.Bacc(None, target_bir_lowering=False)
    f32 = mybir.dt.float32
    f32r = mybir.dt.float32r

    xs = nc.dram_tensor("xs", [ROWS, D], f32, kind="ExternalInput")
    ys = nc.dram_tensor("ys", [ROWS, D], f32, kind="ExternalInput")
    aw = nc.dram_tensor("aw", [MC, G], f32r, kind="ExternalInput")
    c1_part = nc.dram_tensor("c1_part", [128, G], f32, kind="ExternalOutput")
    c2_part = nc.dram_tensor("c2_part", [F, G], f32, kind="ExternalOutput")
    sse_part = nc.dram_tensor("sse_part", [128, NXY], f32, kind="ExternalOutput")

    RW = 256                 # fp32r full-rate moving width
    PAD = RW - G             # 64 junk floats past each block's last strip
    MAXS = max(A_PLAN)

    with tile.TileContext(nc) as tc:
        with (
            tc.tile_pool(name="apool", bufs=4) as apool,
            tc.tile_pool(name="xpool", bufs=1) as xpool,
            tc.tile_pool(name="ypool", bufs=1) as ypool,
            tc.tile_pool(name="dpool", bufs=1) as dpool,
            tc.tile_pool(name="qpool", bufs=1) as qpool,
            tc.tile_pool(name="opool", bufs=1) as opool,
            tc.tile_pool(name="psum", bufs=1, space=bass.MemorySpace.PSUM) as psum,
        ):
            cps1 = psum.tile([128, RW], f32, tag="cps1")
            cps2 = psum.tile([F, RW], f32, tag="cps2")
            # scratch bank for the HAM warm-up burst (never read)
            wps = psum.tile([128, RW], f32, tag="wps")
            acc = opool.tile([128, NXY], f32)

            xv = xs[:].rearrange("(p t) d -> p (t d)", p=128)
            yv = ys[:].rearrange("(p t) d -> p (t d)", p=128)

            acts = []        # deferred (dtile, chunk-idx) Square-accums

            def emit_xy_bulk():
                # x and y as ONE 4MB DMA each (32KB per-partition
                # descriptors): every dma_start boundary costs ~1us of
                # per-engine descriptor-refill stall, so 2 transfers
                # instead of 8 shrink the PE's forced idle window by
                # several us.  MSE runs in two halves off the big tiles.
                # (Moving a y half later into the A stream was tried:
                # the PE lacks the backlog to absorb it - 4.6us gap plus
                # a HAM re-throttle, net wash.)
                xt = xpool.tile([128, NXY * XYW], f32)
                nc.sync.dma_start(xt[:], xv[:])
                yt = ypool.tile([128, NXY * XYW], f32)
                nc.sync.dma_start(yt[:], yv[:])
                for c in range(NXY):
                    dtile = dpool.tile([128, XYW], f32)
                    nc.vector.tensor_sub(
                        dtile[:], xt[:, c * XYW:(c + 1) * XYW],
                        yt[:, c * XYW:(c + 1) * XYW])
                    acts.append((dtile, c))

            # x/y bulk FIRST: the PE must idle for its transfer time
            # no matter what, and before the first A block it has no
            # work to lose; afterwards the A feed (~213ns/strip) stays
            # ahead of the PE (~233ns/strip) so the PE runs gap-free,
            # warm, with no mid-stream restart.  MSE pipelines in the
            # shadow of the A phase.
            emit_xy_bulk()

            # HAM warm-up burst: back-to-back garbage fp32 matmuls on
            # the first d tile fire as soon as sub0 completes (~27us,
            # right after y lands) and keep the PE at 100% duty for
            # ~7us, so it passes the 3.4us HAM window and reaches A0's
            # landing (~34-37us) already at 2.4GHz instead of paying a
            # ~2-3us cold tax on the first real strips.  (Low-duty
            # keep-alive matmuls do NOT work; a contiguous burst does.)
            # fp32 runs at 4cyc/row, so 13 matmuls span the window.
            d0 = acts[0][0]
            for _ in range(13):
                nc.tensor.matmul(wps[:], d0[:, 0:128], d0[:, 0:RW],
                                 start=True, stop=True)

            ti = 0           # global strip counter
            row0 = 0
            for bi, ns in enumerate(A_PLAN):
                at = apool.tile([128, MAXS * G + PAD], f32r)
                src = aw[row0:row0 + 128 * ns].rearrange(
                    "(p t) g -> p (t g)", p=128)
                nc.sync.dma_start(at[:, :ns * G], src)
                nc.gpsimd.memset(
                    at[:, ns * G:ns * G + PAD].bitcast(f32), 0.0)
                row0 += 128 * ns

                for t in range(ns):
                    rhs = at[:, t * G:t * G + RW]
                    w1 = at[:, t * G:t * G + 128]
                    w2 = at[:, t * G + 128:t * G + G]
                    nc.tensor.matmul(
                        cps1[:], w1, rhs,
                        start=(ti == 0), stop=(ti == NSTRIP - 1),
                    )
                    nc.tensor.matmul(
                        cps2[:], w2, rhs,
                        start=(ti == 0), stop=(ti == NSTRIP - 1),
                    )
                    ti += 1

            # MSE squares on ACT run early (inputs land in the xy phase);
            # they must precede the C out-DMAs in the Activation queue,
            # which block on the final matmul.
            for dtile, c in acts:
                qtile = qpool.tile([128, XYW], f32)
                nc.scalar.activation(
                    qtile[:], dtile[:],
                    mybir.ActivationFunctionType.Square,
                    accum_out=acc[:, c:c + 1],
                )
            nc.scalar.dma_start(sse_part[:], acc[:])

            # C partials out on the Activation ring
            o1 = opool.tile([128, G], f32, tag="o1")
            nc.vector.tensor_copy(o1[:], cps1[:, :G])
            nc.scalar.dma_start(c1_part[:], o1[:])
            o2 = opool.tile([F, G], f32, tag="o2")
            nc.vector.tensor_copy(o2[:], cps2[:, :G])
            nc.scalar.dma_start(c2_part[:], o2[:])

    nc.finalize()
    return nc


def _get_prog() -> bass.Bass:
    global _prog
    if _prog is None:
        _prog = _build()
    return _prog


def _epilogue(C: np.ndarray, sse: float) -> np.ndarray:
    # gram[i,j] = sum_k C[3i+k, 3j+k]
    gram = np.einsum("ikjl,kl->ij", C.reshape(F, KW, F, KW), np.eye(KW))
    norms = np.sqrt(np.diag(gram))
    sim = gram / np.outer(norms, norms)
    mask = (sim > TAU) & (sim <= 1.0) & (~np.eye(F, dtype=bool))
    reg = sim[mask].sum()
    loss = sse / float(B * D) + ALPHA * reg
    return np.asarray(np.float32(loss))


def kernel(x_batch: np.ndarray, y_batch: np.ndarray, conv_w: np.ndarray) -> np.ndarray:
    nc = _get_prog()
    A = np.ascontiguousarray(conv_w.reshape(M, G))
    in_maps = []
    for c in range(N_CORES):
        in_maps.append({
            "xs": np.ascontiguousarray(x_batch[c * ROWS:(c + 1) * ROWS]),
            "ys": np.ascontiguousarray(y_batch[c * ROWS:(c + 1) * ROWS]),
            "aw": np.ascontiguousarray(A[c * MC:(c + 1) * MC]),
        })
    res = run_bass_kernel_spmd(nc, in_maps, core_ids=list(range(N_CORES))).results
    C = np.zeros((G, G), np.float64)
    sse = 0.0
    for r in res:
        C[:128] += r["c1_part"].astype(np.float64)
        C[128:] += r["c2_part"].astype(np.float64)
        sse += float(r["sse_part"].sum(dtype=np.float64))
    return _epilogue(C, sse)
